# revision 1
# baseline (speedup 1.0000x reference)
"""Depthwise Conv1d for 8 trn2 cores — channel-major device layout.

Sharding: batch-parallel (B == n_cores == 8). The host transposes each
batch element to channel-major [C, L] before dispatch and transposes the
[C, LOUT] result back afterwards (memory-bandwidth-bound numpy copies,
parallelized across batch elements). On device the depthwise conv is pure
shifted-slice arithmetic with no transposes, no PSUM, no TensorE work:

  xt[128c, W+3]  <- strided DMA from x_t (4-16KB contiguous runs/partition)
  acc  = Identity(xt[:, 0:W]*w0 + bias)   ACT, per-partition scale+bias
  acc  = (xt[:, k:k+W]*wk) + acc          DVE scalar_tensor_tensor, k=1,2,3
  out_t[cb, o0:o0+W] <- DMA acc

DVE is the critical path (3 fused multiply-add passes at fp32 1x mode,
~218us/core) slightly above the ~203us HBM-DMA floor; ACT carries tap 0.
GPSIMD stays idle on purpose: its SBUF access arbitrates an exclusive
port-pair lock against DVE 2-input ops and stalls both engines.
"""

import sys
from concurrent.futures import ThreadPoolExecutor

for _p in ("/opt/trn_rl_repo", "/root/.axon_site/_ro/trn_rl_repo"):
    if _p not in sys.path:
        sys.path.insert(0, _p)

import numpy as np

import concourse.bass as bass  # noqa: F401
import concourse.tile as tile
from concourse import bacc, mybir
from concourse.bass_utils import run_bass_kernel_spmd

F32 = mybir.dt.float32
MULT = mybir.AluOpType.mult
ADD = mybir.AluOpType.add
COPY = mybir.ActivationFunctionType.Copy
IDENT = mybir.ActivationFunctionType.Identity

B, L, C, K, PAD = 8, 4096, 2048, 4, 3
LOUT = L + 2 * PAD - K + 1  # 4099
NCB = C // 128  # 16 channel blocks
CHUNK = 2048
NCHUNK = 2  # chunk 0: [0,2048); chunk 1: [2048, 4099)
TAIL = LOUT - NCHUNK * CHUNK  # 3

# GPSIMD compute is intentionally unused: any GPSIMD tensor op arbitrates an
# exclusive SBUF port-pair lock against DVE 2-input ops, stalling both.
ACT_TAP1_CBS = 0


def _build_nc():
    nc = bacc.Bacc("TRN2", target_bir_lowering=False, num_devices=B)

    xt_d = nc.dram_tensor("xt", [C, L], F32, kind="ExternalInput")
    wt_d = nc.dram_tensor("wt", [128, NCB * K], F32, kind="ExternalInput")
    bt_d = nc.dram_tensor("bt", [128, NCB], F32, kind="ExternalInput")
    out_d = nc.dram_tensor("out", [C, LOUT], F32, kind="ExternalOutput")

    with tile.TileContext(nc) as tc:
        with (
            tc.tile_pool(name="const", bufs=1) as cpool,
            tc.tile_pool(name="xt", bufs=5) as xt_pool,
            tc.tile_pool(name="acc", bufs=5) as acc_pool,
        ):
            wt_sb = cpool.tile([128, NCB * K], F32)
            bt_sb = cpool.tile([128, NCB], F32)

            # tiny Identity activation up front so ACT_TABLE_LOAD happens
            # during the DMA ramp, not in front of the first real tap-0 op
            warm = cpool.tile([128, 1], F32)
            nc.vector.memset(warm[:], 0.0)
            nc.scalar.activation(
                out=warm[:], in_=warm[:], func=IDENT, scale=1.0
            )

            def load_xt(cb, o0, width):
                cs = slice(cb * 128, (cb + 1) * 128)
                xtw = width + PAD  # needs x cols [o0-3, o0+width)
                lo = o0 - PAD
                hi = min(o0 + width, L)
                xt = xt_pool.tile([128, xtw], F32, tag="xt")
                if o0 == 0:
                    nc.vector.memset(xt[:, 0:PAD], 0.0)
                    nc.sync.dma_start(
                        out=xt[:, PAD : PAD + hi], in_=xt_d[cs, 0:hi]
                    )
                else:
                    nc.sync.dma_start(
                        out=xt[:, 0 : hi - lo], in_=xt_d[cs, lo:hi]
                    )
                if hi - lo < xtw:
                    # zero-pad for virtual x rows beyond L
                    nc.vector.memset(xt[:, hi - lo : xtw], 0.0)
                return xt

            def emit_unit(cb, o0, width, xt=None):
                """Produce out[cb*128:(cb+1)*128, o0:o0+width]."""
                cs = slice(cb * 128, (cb + 1) * 128)
                if xt is None:
                    xt = load_xt(cb, o0, width)

                wk = lambda k: wt_sb[:, cb * K + k : cb * K + k + 1]

                acc = acc_pool.tile([128, width], F32, tag="acc")
                nc.scalar.activation(
                    out=acc[:],
                    in_=xt[:, 0:width],
                    func=IDENT,
                    scale=wk(0),
                    bias=bt_sb[:, cb : cb + 1],
                )
                for k in (1, 2, 3):
                    nc.vector.scalar_tensor_tensor(
                        out=acc[:],
                        in0=xt[:, k : k + width],
                        scalar=wk(k),
                        in1=acc[:],
                        op0=MULT,
                        op1=ADD,
                    )
                nc.sync.dma_start(out=out_d[cs, o0 : o0 + width], in_=acc[:])

            # small units first to prime the pipeline and last to drain it
            # quickly; full-L units in the middle. The first unit's input DMA
            # is issued before the (tiny) weight/bias DMAs to cut the ramp.
            units = [
                (0, 0, 1024), (0, 1024, 1024), (0, 2048, 1024), (0, 3072, 1027),
                (1, 0, 2048), (1, 2048, 2051),
            ]
            units += [(cb, 0, LOUT) for cb in range(2, NCB - 1)]
            units += [(NCB - 1, 0, 2048), (NCB - 1, 2048, 2051)]
            xt0 = load_xt(*units[0])
            nc.sync.dma_start(out=wt_sb[:], in_=wt_d[:])
            nc.sync.dma_start(out=bt_sb[:], in_=bt_d[:])
            emit_unit(*units[0], xt=xt0)
            for cb, o0, width in units[1:]:
                emit_unit(cb, o0, width)

    nc.compile()
    return nc


_NC_CACHE = None


def _get_nc():
    global _NC_CACHE
    if _NC_CACHE is None:
        _NC_CACHE = _build_nc()
    return _NC_CACHE


def _const_inputs(weight, bias):
    wt = np.ascontiguousarray(
        weight.astype(np.float32).reshape(NCB, 128, K).transpose(1, 0, 2)
    ).reshape(128, NCB * K)
    bt = np.ascontiguousarray(bias.astype(np.float32).reshape(NCB, 128).T)
    return wt, bt


def _in_maps(x, weight, bias):
    wt, bt = _const_inputs(weight, bias)
    with ThreadPoolExecutor(max_workers=8) as ex:
        xts = list(ex.map(lambda b: np.ascontiguousarray(x[b].T), range(B)))
    return [{"xt": xts[b], "wt": wt, "bt": bt} for b in range(B)]


def kernel(x, weight, bias):
    x = np.asarray(x)
    weight = np.asarray(weight)
    bias = np.asarray(bias)
    assert x.shape == (B, L, C) and weight.shape == (C, K) and bias.shape == (C,)
    nc = _get_nc()
    in_maps = _in_maps(x, weight, bias)
    res = run_bass_kernel_spmd(nc, in_maps, core_ids=list(range(B)))

    out = np.empty((B, LOUT, C), dtype=np.float32)
    with ThreadPoolExecutor(max_workers=8) as ex:
        list(
            ex.map(
                lambda b: np.copyto(out[b], res.results[b]["out"].T), range(B)
            )
        )
    return out


if __name__ == "__main__":
    rng = np.random.default_rng(0)
    x = rng.standard_normal((B, L, C), dtype=np.float32)
    w = (rng.standard_normal((C, K)) * 0.1).astype(np.float32)
    bias = (rng.standard_normal((C,)) * 0.1).astype(np.float32)
    out = kernel(x, w, bias)
    print("out", out.shape, out.dtype)



# revision 3
# speedup vs baseline: 1.0023x; 1.0023x over previous
"""Depthwise Conv1d on 8 trn2 cores — bf16 I/O + TensorE banded-diagonal matmuls.

Sharding: channel-parallel (256 of 2048 channels per core, all 8 batch
elements). The host packs each channel's padded input into 4 L-phases
(xpad[4u+q], u-major) so a partition block holds 32 channels x 4 phases;
then the K=4 depthwise taps collapse into TWO matmuls per 512-col PSUM
chunk (shift-0/shift-1 moving views x banded stationary weights):

  psum[(c,p), j] = W0.T @ XP[:, j] + W1.T @ XP[:, j+1]
  W0[(c,q),(c,p)] = w[c, q-p]   (q >= p)
  W1[(c,q),(c,p)] = w[c, q-p+4] (q <  p)

PE accumulates all taps in PSUM fp32 (~108us/core incl the ~163ns/matmul
fixed cost and HAM duty-cycle throttle); DVE/ACT only evacuate PSUM ->
SBUF with bias add + bf16 cast (~48/44us); DMA moves 34 MB/core bf16
(~107us at the throttled ~320 GB/s). PE and DMA are co-critical; bf16
halves HBM traffic vs fp32 and keeps rel err ~2.7e-3 (gate 2e-2). The
l in [4096, 4099) output tail (3 cols) is computed on host in fp32.
Measured: 127.9us vs 238-283us for the fp32 DVE-only baseline.
"""

import sys
from concurrent.futures import ThreadPoolExecutor

for _p in ("/opt/trn_rl_repo", "/root/.axon_site/_ro/trn_rl_repo"):
    if _p not in sys.path:
        sys.path.insert(0, _p)

import ml_dtypes
import numpy as np

import concourse.bass as bass  # noqa: F401
import concourse.tile as tile
from concourse import bacc, mybir
from concourse.bass_utils import run_bass_kernel_spmd

F32 = mybir.dt.float32
BF16 = mybir.dt.bfloat16
ADD = mybir.AluOpType.add
IDENT = mybir.ActivationFunctionType.Identity
NPBF16 = ml_dtypes.bfloat16

B, L, C, K, PAD = 8, 4096, 2048, 4, 3
LOUT = L + 2 * PAD - K + 1  # 4099
NCORE = 8
CPC = C // NCORE  # 256 channels per core
NCG = CPC // 32  # 8 channel groups (32ch x 4phase = 128 partitions)
U = 1026  # phase columns incl halo/pad
J = 1024  # output columns per phase row (l=4096..4098 tail done on host)
CHUNKS = ((0, 512), (512, 512))


def _build_nc():
    nc = bacc.Bacc("TRN2", target_bir_lowering=False, num_devices=NCORE)

    xp_d = nc.dram_tensor("xp", [NCG * 128, B * U], BF16, kind="ExternalInput")
    wd_d = nc.dram_tensor("wd", [128, NCG * 256], BF16, kind="ExternalInput")
    bd_d = nc.dram_tensor("bd", [128, NCG], F32, kind="ExternalInput")
    out_d = nc.dram_tensor("out", [NCG * 128, B * J], BF16, kind="ExternalOutput")

    with tile.TileContext(nc) as tc:
        with (
            tc.tile_pool(name="const", bufs=1) as cpool,
            tc.tile_pool(name="x", bufs=4) as xpool,
            tc.tile_pool(name="o", bufs=4) as opool,
            tc.tile_pool(name="psum", bufs=6, space="PSUM") as ppool,
        ):
            wt = cpool.tile([128, NCG * 256], BF16)
            bt = cpool.tile([128, NCG], F32)

            # tiny Identity activation up front so ACT_TABLE_LOAD happens
            # during the DMA ramp, not in front of the first real evac
            warm = cpool.tile([128, 1], F32)
            nc.vector.memset(warm[:], 0.0)
            nc.scalar.activation(out=warm[:], in_=warm[:], func=IDENT, scale=1.0)

            def do_pair(cg, bp, first=False):
                cs = slice(cg * 128, (cg + 1) * 128)
                xt = xpool.tile([128, 2 * U], BF16, tag="x")
                nc.sync.dma_start(
                    out=xt[:], in_=xp_d[cs, bp * 2 * U : (bp + 1) * 2 * U]
                )
                if first:
                    # issue after the first x tile so the ramp starts on x
                    nc.sync.dma_start(out=wt[:], in_=wd_d[:])
                    nc.sync.dma_start(out=bt[:], in_=bd_d[:])
                w0 = wt[:, cg * 256 : cg * 256 + 128]
                w1 = wt[:, cg * 256 + 128 : cg * 256 + 256]
                bb = bt[:, cg : cg + 1]
                ot = opool.tile([128, 2 * J], BF16, tag="o")
                for r in (0, 1):
                    # W0 pass over both chunks, then W1 pass: 2 LDWEIGHTS
                    # per (cg, b) instead of one per matmul
                    pss = [
                        ppool.tile([128, 512], F32, tag="ps", name=f"ps_{ci}")
                        for ci in range(len(CHUNKS))
                    ]
                    for ps, (j0, width) in zip(pss, CHUNKS):
                        x0 = xt[:, r * U + j0 : r * U + j0 + width]
                        nc.tensor.matmul(ps[:], w0, x0, start=True, stop=False)
                    for ci, (ps, (j0, width)) in enumerate(zip(pss, CHUNKS)):
                        x1 = xt[:, r * U + j0 + 1 : r * U + j0 + 1 + width]
                        nc.tensor.matmul(ps[:], w1, x1, start=False, stop=True)
                        osl = ot[:, r * J + j0 : r * J + j0 + width]
                        if ci == 1:
                            nc.scalar.activation(
                                out=osl, in_=ps[:], func=IDENT, scale=1.0, bias=bb
                            )
                        else:
                            nc.vector.tensor_scalar(osl, ps[:], bb, None, ADD)
                nc.sync.dma_start(
                    out=out_d[cs, bp * 2 * J : (bp + 1) * 2 * J], in_=ot[:]
                )

            first = True
            for cg in range(NCG):
                for bp in range(B // 2):
                    do_pair(cg, bp, first=first)
                    first = False

    nc.compile()
    return nc


_NC_CACHE = None


def _get_nc():
    global _NC_CACHE
    if _NC_CACHE is None:
        _NC_CACHE = _build_nc()
    return _NC_CACHE


def _pack_core(x, weight, bias, core):
    sl = slice(core * CPC, (core + 1) * CPC)
    xc = x[:, :, sl].astype(NPBF16)  # (B, L, 256)
    t = np.ascontiguousarray(xc.transpose(2, 0, 1))  # (256, B, L)
    Q = np.zeros((4, CPC, B, U), dtype=NPBF16)
    for q in range(3):
        Q[q][:, :, 1:1025] = t[:, :, (q + 1) :: 4]
    Q[3][:, :, 0:1024] = t[:, :, 0::4]
    xp = np.ascontiguousarray(Q.transpose(1, 0, 2, 3)).reshape(NCG * 128, B * U)

    wc = weight[sl].astype(NPBF16)  # (256, K)
    w_host = np.zeros((128, NCG * 256), dtype=NPBF16)
    cl = np.arange(32)
    for cg in range(NCG):
        cglob = cg * 32 + cl
        for p in range(4):
            for q in range(4):
                k = q - p
                if 0 <= k < K:
                    w_host[cl * 4 + q, cg * 256 + cl * 4 + p] = wc[cglob, k]
                k4 = q - p + 4
                if q < p and k4 < K:
                    w_host[cl * 4 + q, cg * 256 + 128 + cl * 4 + p] = wc[cglob, k4]

    bc = bias[sl].astype(np.float32)
    b_host = np.zeros((128, NCG), dtype=np.float32)
    for p in range(4):
        b_host[cl * 4 + p, :] = bc.reshape(NCG, 32).T[cl, :]
    return {"xp": xp, "wd": w_host, "bd": b_host}


def _in_maps(x, weight, bias):
    with ThreadPoolExecutor(max_workers=NCORE) as ex:
        return list(
            ex.map(lambda i: _pack_core(x, weight, bias, i), range(NCORE))
        )


def _unpack_core(res, out, core):
    sl = slice(core * CPC, (core + 1) * CPC)
    v = np.asarray(res["out"]).reshape(NCG, 32, 4, B, J)
    o = np.ascontiguousarray(v.transpose(3, 4, 2, 0, 1)).reshape(B, 4 * J, CPC)
    out[:, : 4 * J, sl] = o.astype(np.float32)


def _tail_cols(x, weight, bias, out):
    # device computes l in [0, 4096); the last 3 output columns come from
    # x[4093:] only — do them on host in fp32
    for r in range(LOUT - 4 * J):
        acc = np.broadcast_to(bias[None, :], (B, C)).copy()
        for k in range(K):
            idx = 4 * J + r + k - PAD
            if idx < L:
                acc += x[:, idx, :] * weight[None, :, k]
        out[:, 4 * J + r, :] = acc


def kernel(x, weight, bias):
    x = np.asarray(x)
    weight = np.asarray(weight)
    bias = np.asarray(bias)
    assert x.shape == (B, L, C) and weight.shape == (C, K) and bias.shape == (C,)
    nc = _get_nc()
    in_maps = _in_maps(x, weight, bias)
    res = run_bass_kernel_spmd(nc, in_maps, core_ids=list(range(NCORE)))

    out = np.empty((B, LOUT, C), dtype=np.float32)
    with ThreadPoolExecutor(max_workers=NCORE) as ex:
        list(
            ex.map(
                lambda i: _unpack_core(res.results[i], out, i), range(NCORE)
            )
        )
    _tail_cols(x, weight, bias, out)
    return out


if __name__ == "__main__":
    rng = np.random.default_rng(0)
    x = rng.standard_normal((B, L, C), dtype=np.float32)
    w = (rng.standard_normal((C, K)) * 0.1).astype(np.float32)
    bias = (rng.standard_normal((C,)) * 0.1).astype(np.float32)
    out = kernel(x, w, bias)
    print("out", out.shape, out.dtype)
